# revision 1
# baseline (speedup 1.0000x reference)
"""DiGCN_IB_1BN kernel for Trainium2 (8 NeuronCores, SPMD data-parallel).

Math (see reference):
  out = BN(x @ Wl + bl + conv1 + conv2)
  conv_g = segment_sum((x @ Wg)[src] * w, dst) + bg, edges masked to
  same-1024-block pairs only.

Strategy:
  - BN + biases folded on host into per-channel scale (into the W mats) and
    a single additive shift.
  - Nodes (and their incident same-block edges) sharded across 8 cores by
    contiguous 13-block groups (13*1024 = 13312 nodes/core). Zero cross-core
    communication.
  - On device, per core:
      dense:    h0 = xT_c.T @ Wl'  (+shift) -> out rows (direct DMA store)
      messages: per 128-token tile, msg = xTe_tile.T @ Wg' scaled by per-edge
                weight (tensor_scalar), xTe holds x[src] columns pre-gathered
                on host in token order.
      scatter:  dma_scatter_add (SWDGE CCE-add) of msg rows into out rows at
                dst. Tokens are grouped into "rounds" with unique dst within
                each round; rounds are separate DMA instructions that Tile
                serializes (RMW-add races would otherwise lose updates).
"""

import sys

sys.path.insert(0, "/opt/trn_rl_repo")

from contextlib import ExitStack

import numpy as np

import concourse.bass as bass
import concourse.tile as tile
from concourse import bacc, mybir
from concourse._compat import with_exitstack
from concourse.bass_utils import run_bass_kernel_spmd

# problem constants (hardcoded per harness contract)
N = 100000
F = 128
C = 64
BS = 1024
EPS = 1e-5
NCORES = 8
BPC = 13  # blocks per core
NC_NODES = BPC * BS  # 13312
NPAD = NCORES * NC_NODES  # 106496
P = 128
N_OUT = NC_NODES + P  # + dump rows for padding tokens


def _prep(x, edge_index, edge_weight, edge_index2, edge_weight2,
          Wl, bl, W1, b1, W2, b2, gamma, beta, run_mean, run_var):
    """Host-side sharding + layout. Returns (in_maps, static_cfg)."""
    inv = (gamma / np.sqrt(run_var + EPS)).astype(np.float32)
    Wcat = np.concatenate(
        [Wl * inv[None, :], W1 * inv[None, :], W2 * inv[None, :]], axis=1
    ).astype(np.float32)  # [128, 192]
    shift = ((bl + b1 + b2 - run_mean) * inv + beta).astype(np.float32)
    shift_rep = np.ascontiguousarray(np.tile(shift[None, :], (P, 1)))

    xpad = np.zeros((NPAD, F), np.float32)
    xpad[:N] = x

    # per-core, per-graph surviving edges (local indices)
    cores = []
    for c in range(NCORES):
        cores.append({"src": [], "dst": [], "w": []})
    for g, (ei, ew) in enumerate([(edge_index, edge_weight),
                                  (edge_index2, edge_weight2)]):
        src = np.asarray(ei[0], dtype=np.int64)
        dst = np.asarray(ei[1], dtype=np.int64)
        keep = (src // BS) == (dst // BS)
        src = src[keep]
        dst = dst[keep]
        w = np.asarray(ew, dtype=np.float32)[keep]
        core = dst // NC_NODES
        for c in range(NCORES):
            m = core == c
            cores[c]["src"].append((src[m] - c * NC_NODES).astype(np.int32))
            cores[c]["dst"].append((dst[m] - c * NC_NODES).astype(np.int32))
            cores[c]["w"].append(w[m])

    # round assignment: occurrence index per dst across both graphs
    # per core: occs[c][g] = occurrence index of each edge
    all_occ = []
    for c in range(NCORES):
        cnt = np.zeros(NC_NODES, np.int32)
        occ_per_g = []
        for g in range(2):
            d = cores[c]["dst"][g]
            occ = np.empty(len(d), np.int32)
            for i, dd in enumerate(d):
                occ[i] = cnt[dd]
                cnt[dd] += 1
            occ_per_g.append(occ)
        all_occ.append(occ_per_g)

    R = 1
    for c in range(NCORES):
        for g in range(2):
            if len(all_occ[c][g]):
                R = max(R, int(all_occ[c][g].max()) + 1)

    # static segment sizes: k[r][g] slots (multiples of 128 tokens)
    kseg = np.zeros((R, 2), np.int32)
    for r in range(R):
        for g in range(2):
            mx = 0
            for c in range(NCORES):
                mx = max(mx, int((all_occ[c][g] == r).sum()))
            kseg[r, g] = max(1, -(-mx // P))  # at least 1 slot to keep layout simple
    S_TOT = int(kseg.sum())
    NTOK = S_TOT * P

    # build per-core arrays
    in_maps = []
    for c in range(NCORES):
        src_tok = np.zeros(NTOK, np.int32)  # local node idx of token's source
        w_tok = np.zeros(NTOK, np.float32)
        dst_tok = np.full(NTOK, -1, np.int32)  # -1 => pad (dump row)
        s0 = 0
        for r in range(R):
            for g in range(2):
                m = all_occ[c][g] == r
                n = int(m.sum())
                j0 = s0 * P
                src_tok[j0:j0 + n] = cores[c]["src"][g][m]
                dst_tok[j0:j0 + n] = cores[c]["dst"][g][m]
                w_tok[j0:j0 + n] = cores[c]["w"][g][m]
                s0 += int(kseg[r, g])
        assert s0 == S_TOT

        # token j lives at sbuf position [j%128, j//128]
        xTe = np.ascontiguousarray(
            xpad[c * NC_NODES + src_tok].T
        )  # [128, NTOK] fp32 (pad tokens read node 0; w=0 kills them)
        w_col = np.ascontiguousarray(w_tok.reshape(S_TOT, P).T)  # [128, S_TOT]
        idx_val = dst_tok.copy()
        pads = idx_val < 0
        idx_val[pads] = NC_NODES + (np.arange(NTOK)[pads] % P)  # dump rows
        idxs16 = np.zeros((P, NTOK // 16), np.int16)
        base = idx_val.reshape(NTOK // 16, 16).T.astype(np.int16)  # [16, NTOK/16]
        for rep in range(8):
            idxs16[rep * 16:(rep + 1) * 16, :] = base

        xT = np.ascontiguousarray(xpad[c * NC_NODES:(c + 1) * NC_NODES].T)

        in_maps.append({
            "xt": xT,                  # [128, 13312]
            "xte": xTe,                # [128, NTOK]
            "wcat": Wcat,              # [128, 192]
            "shift": shift_rep,        # [128, 64]
            "wcol": w_col,             # [128, S_TOT]
            "idxs": idxs16,            # [128, NTOK/16]
        })

    # rounds: (slot_lo, n_slots, graph) segments and per-round slot spans
    segs = []  # (slot0, nslots, graph) in emission order
    rounds = []  # (slot0, nslots) per scatter instruction
    s0 = 0
    for r in range(R):
        r0 = s0
        for g in range(2):
            segs.append((s0, int(kseg[r, g]), g))
            s0 += int(kseg[r, g])
        rounds.append((r0, s0 - r0))

    cfg = {"S_TOT": S_TOT, "segs": segs, "rounds": rounds}
    return in_maps, cfg


@with_exitstack
def _emit(ctx: ExitStack, tc: tile.TileContext, io, cfg):
    nc = tc.nc
    out_d = io["out"]
    S_TOT = cfg["S_TOT"]
    XCHUNK = 13 * P  # 1664 cols (~0.85MB) per xt load chunk
    ECHUNK = 8  # slots per xte load chunk (0.5MB)

    const = ctx.enter_context(tc.tile_pool(name="const", bufs=1))
    osb = ctx.enter_context(tc.tile_pool(name="osb", bufs=8))
    ps = ctx.enter_context(tc.tile_pool(name="ps", bufs=8, space="PSUM"))

    W_sb = const.tile([P, 3 * C], mybir.dt.float32)
    nc.sync.dma_start(W_sb[:], io["wcat"][:])
    shift_sb = const.tile([P, C], mybir.dt.float32)
    nc.sync.dma_start(shift_sb[:], io["shift"][:])
    w_sb = const.tile([P, S_TOT], mybir.dt.float32)
    nc.sync.dma_start(w_sb[:], io["wcol"][:])
    idx_sb = const.tile([P, S_TOT * 8], mybir.dt.int16)
    nc.sync.dma_start(idx_sb[:], io["idxs"][:])

    xte_sb = const.tile([P, S_TOT * P], mybir.dt.float32)
    for lo in range(0, S_TOT, ECHUNK):
        hi = min(lo + ECHUNK, S_TOT)
        nc.sync.dma_start(xte_sb[:, lo * P:hi * P], io["xte"][:, lo * P:hi * P])

    xt_sb = const.tile([P, NC_NODES], mybir.dt.float32)
    for lo in range(0, NC_NODES, XCHUNK):
        hi = min(lo + XCHUNK, NC_NODES)
        nc.sync.dma_start(xt_sb[:, lo:hi], io["xt"][:, lo:hi])

    # message phase (emitted first so PE fills msg tiles early; scatters can
    # only run after dense stores anyway, but msg tiles gate nothing else)
    msg_sb = const.tile([P, S_TOT, C], mybir.dt.float32)
    for (slot0, nslots, g) in cfg["segs"]:
        for s in range(slot0, slot0 + nslots):
            psum = ps.tile([P, C], mybir.dt.float32)
            nc.tensor.matmul(
                psum[:],
                lhsT=xte_sb[:, s * P:(s + 1) * P],
                rhs=W_sb[:, (1 + g) * C:(2 + g) * C],
                start=True,
                stop=True,
            )
            nc.vector.tensor_scalar(
                out=msg_sb[:, s, :],
                in0=psum[:],
                scalar1=w_sb[:, s:s + 1],
                scalar2=None,
                op0=mybir.AluOpType.mult,
            )

    # dense phase
    for t in range(NC_NODES // P):
        psum = ps.tile([P, C], mybir.dt.float32)
        nc.tensor.matmul(
            psum[:],
            lhsT=xt_sb[:, t * P:(t + 1) * P],
            rhs=W_sb[:, 0:C],
            start=True,
            stop=True,
        )
        o_sb = osb.tile([P, C], mybir.dt.float32)
        nc.vector.tensor_add(o_sb[:], psum[:], shift_sb[:])
        nc.sync.dma_start(out_d[t * P:(t + 1) * P, :], o_sb[:])

    # scatter rounds (Tile serializes via WAW on out_d)
    for (slot0, nslots) in cfg["rounds"]:
        n = nslots * P
        nc.gpsimd.dma_scatter_add(
            out_d[:, :],
            msg_sb[:, slot0:slot0 + nslots, :],
            idx_sb[:, slot0 * 8:(slot0 + nslots) * 8],
            n,
            n,
            C,
        )


def _build(cfg):
    nc = bacc.Bacc("TRN2", target_bir_lowering=False, debug=False)
    io = {}
    io["xt"] = nc.dram_tensor("xt", [P, NC_NODES], mybir.dt.float32,
                              kind="ExternalInput").ap()
    io["xte"] = nc.dram_tensor("xte", [P, cfg["S_TOT"] * P], mybir.dt.float32,
                               kind="ExternalInput").ap()
    io["wcat"] = nc.dram_tensor("wcat", [P, 3 * C], mybir.dt.float32,
                                kind="ExternalInput").ap()
    io["shift"] = nc.dram_tensor("shift", [P, C], mybir.dt.float32,
                                 kind="ExternalInput").ap()
    io["wcol"] = nc.dram_tensor("wcol", [P, cfg["S_TOT"]], mybir.dt.float32,
                                kind="ExternalInput").ap()
    io["idxs"] = nc.dram_tensor("idxs", [P, cfg["S_TOT"] * 8], mybir.dt.int16,
                                kind="ExternalInput").ap()
    io["out"] = nc.dram_tensor("out", [N_OUT, C], mybir.dt.float32,
                               kind="ExternalOutput").ap()
    with tile.TileContext(nc) as tc:
        _emit(tc, io, cfg)
    nc.compile()
    return nc


def kernel(_trace=False, _sim_core=None, **inputs) -> np.ndarray:
    in_maps, cfg = _prep(**inputs)
    nc = _build(cfg)

    if _sim_core is not None:
        # CoreSim single-core validation path (returns that core's raw out)
        from concourse.bass_interp import CoreSim
        sim = CoreSim(nc, trace=False)
        for k, v in in_maps[_sim_core].items():
            sim.tensor(k)[:] = v
        sim.tensor("out")[:] = 0.0
        sim.simulate(check_with_hw=False)
        return np.array(sim.tensor("out"))

    res = run_bass_kernel_spmd(
        nc, in_maps, core_ids=list(range(NCORES)),
        trace=_trace, trace_cores=[0] if _trace else None,
    )
    out = np.empty((NPAD, C), np.float32)
    for c in range(NCORES):
        out[c * NC_NODES:(c + 1) * NC_NODES] = res.results[c]["out"][:NC_NODES]
    if _trace:
        kernel.last_exec_time_ns = res.exec_time_ns
        kernel.last_results = res
    return out[:N]


# revision 3
# speedup vs baseline: 2.1879x; 2.1879x over previous
"""DiGCN_IB_1BN kernel for Trainium2 (8 NeuronCores, SPMD data-parallel).

Math (see reference):
  out = BN(x @ Wl + bl + conv1 + conv2)
  conv_g = segment_sum((x @ Wg)[src] * w, dst) + bg, edges masked to
  same-1024-block pairs only.

Strategy:
  - BN + biases folded on host into per-channel scale (into the W mats) and a
    single additive shift.
  - Nodes (and their incident same-block edges) sharded across 8 cores by
    contiguous 13-block groups (13*1024 = 13312 nodes/core). Zero cross-core
    communication. fp16 on-device matmul inputs (PSUM accumulates fp32).
  - Per core, tokens (surviving edges) are grouped by destination 128-node
    out-tile; one 128-token slot per tile (rarely 2 on overflow). For each
    tile the kernel computes, fully on-chip:
      msg:    psum_m = xte_slot.T @ [W1'|W2']   (both graphs' h for the
              token's source node; per-token graph selection happens via the
              weighted selection matrix)
      S_g:    S_g[k, m] = (dstv[k] == m) * w_g[k]  -- one fused
              tensor_scalar(is_equal, mult) over a constant iota tile; w_g is
              the edge weight if token k belongs to graph g else 0.
      out:    psum_t = xt_tile.T @ Wl' + S_1.T @ msg[:, :64]
                       + S_2.T @ msg[:, 64:]        (PSUM accumulation)
      store:  out_sb = psum_t + shift (DVE) -> contiguous 32KB DMA store.
  No indirect/scatter DMA anywhere (v1's dma_scatter_add measured ~7ns/token
  of serialized Q7 descriptor-gen -- ~100us; the selection-matmul merge
  replaces it with ~300 tiny matmuls).
"""

import sys

sys.path.insert(0, "/opt/trn_rl_repo")

from contextlib import ExitStack

import numpy as np

import concourse.bass as bass
import concourse.tile as tile
from concourse import bacc, mybir
from concourse._compat import with_exitstack
from concourse.bass_utils import run_bass_kernel_spmd

# problem constants (hardcoded per harness contract)
N = 100000
F = 128
C = 64
BS = 1024
EPS = 1e-5
NCORES = 8
BPC = 13  # blocks per core
NC_NODES = BPC * BS  # 13312
NPAD = NCORES * NC_NODES  # 106496
P = 128
NTILES = NC_NODES // P  # 104


def _prep(x, edge_index, edge_weight, edge_index2, edge_weight2,
          Wl, bl, W1, b1, W2, b2, gamma, beta, run_mean, run_var):
    """Host-side sharding + layout. Returns (in_maps, cfg)."""
    inv = (gamma / np.sqrt(run_var + EPS)).astype(np.float32)
    Wcat = np.concatenate(
        [Wl * inv[None, :], W1 * inv[None, :], W2 * inv[None, :]], axis=1
    ).astype(np.float16)  # [128, 192]
    shift = ((bl + b1 + b2 - run_mean) * inv + beta).astype(np.float32)
    shift_rep = np.ascontiguousarray(np.tile(shift[None, :], (P, 1)))
    iota = np.ascontiguousarray(
        np.tile(np.arange(P, dtype=np.float16)[None, :], (P, 1))
    )

    xpad = np.zeros((NPAD, F), np.float16)
    xpad[:N] = x

    # per-core, per-graph surviving edges (local indices), grouped by dst tile
    per_core = [[None, None] for _ in range(NCORES)]
    for g, (ei, ew) in enumerate([(edge_index, edge_weight),
                                  (edge_index2, edge_weight2)]):
        src = np.asarray(ei[0], dtype=np.int64)
        dst = np.asarray(ei[1], dtype=np.int64)
        keep = (src // BS) == (dst // BS)
        src = src[keep]
        dst = dst[keep]
        w = np.asarray(ew, dtype=np.float32)[keep]
        core = dst // NC_NODES
        for c in range(NCORES):
            m = core == c
            per_core[c][g] = (
                (src[m] - c * NC_NODES).astype(np.int32),
                (dst[m] - c * NC_NODES).astype(np.int32),
                w[m],
            )

    # tokens per (core, tile): counts to size slots (static per compile)
    counts = np.zeros((NCORES, NTILES), np.int64)
    for c in range(NCORES):
        for g in range(2):
            dst = per_core[c][g][1]
            np.add.at(counts[c], dst // P, 1)
    slots_per_tile = np.maximum(1, -(-counts.max(axis=0) // P))  # [NTILES]
    # slot layout: tile t owns slots slot0[t] .. slot0[t]+slots_per_tile[t]-1
    slot0 = np.concatenate([[0], np.cumsum(slots_per_tile)])
    NSLOT = int(slot0[-1])

    in_maps = []
    for c in range(NCORES):
        src_tok = np.zeros(NSLOT * P, np.int64)
        dstv = np.zeros(NSLOT * P, np.float32)  # dst - t*128, in [0,128)
        w1t = np.zeros(NSLOT * P, np.float32)
        w2t = np.zeros(NSLOT * P, np.float32)
        fill = np.zeros(NTILES, np.int64)  # tokens placed per tile
        for g in range(2):
            src, dst, w = per_core[c][g]
            t_of = dst // P
            order = np.argsort(t_of, kind="stable")
            for i in order:
                t = t_of[i]
                j = slot0[t] * P + fill[t]  # token position
                fill[t] += 1
                src_tok[j] = src[i]
                dstv[j] = np.float32(dst[i] - t * P)
                (w1t if g == 0 else w2t)[j] = w[i]

        xTe = np.ascontiguousarray(xpad[c * NC_NODES + src_tok].T)  # [128, NSLOT*128] f16
        # token j of slot s sits at partition j%128 => column-major per slot
        dstv_c = np.ascontiguousarray(dstv.reshape(NSLOT, P).T)  # [128, NSLOT]
        w1c = np.ascontiguousarray(w1t.reshape(NSLOT, P).T)
        w2c = np.ascontiguousarray(w2t.reshape(NSLOT, P).T)
        xT = np.ascontiguousarray(xpad[c * NC_NODES:(c + 1) * NC_NODES].T)

        in_maps.append({
            "xt": xT,          # [128, 13312] f16
            "xte": xTe,        # [128, NSLOT*128] f16
            "wcat": Wcat,      # [128, 192] f16
            "shift": shift_rep,  # [128, 64] f32
            "iota": iota,      # [128, 128] f16
            "dstv": dstv_c,    # [128, NSLOT] f32
            "w1": w1c,         # [128, NSLOT] f32
            "w2": w2c,         # [128, NSLOT] f32
        })

    cfg = {"NSLOT": NSLOT,
           "tile_slots": [(t, list(range(int(slot0[t]), int(slot0[t + 1]))))
                          for t in range(NTILES)]}
    return in_maps, cfg


@with_exitstack
def _emit(ctx: ExitStack, tc: tile.TileContext, io, cfg):
    nc = tc.nc
    out_d = io["out"]
    NSLOT = cfg["NSLOT"]
    f16 = mybir.dt.float16
    f32 = mybir.dt.float32
    XCHUNK = 13 * P  # 1664 cols per xt load chunk (~0.85MB f16->0.43MB)
    ECHUNK = 13  # slots per xte load chunk

    const = ctx.enter_context(tc.tile_pool(name="const", bufs=1))
    osb = ctx.enter_context(tc.tile_pool(name="osb", bufs=8))
    spool = ctx.enter_context(tc.tile_pool(name="spool", bufs=8))
    ps = ctx.enter_context(tc.tile_pool(name="ps", bufs=4, space="PSUM"))
    psm = ctx.enter_context(tc.tile_pool(name="psm", bufs=4, space="PSUM"))

    W_sb = const.tile([P, 3 * C], f16)
    nc.sync.dma_start(W_sb[:], io["wcat"][:])
    shift_sb = const.tile([P, C], f32)
    nc.sync.dma_start(shift_sb[:], io["shift"][:])
    iota_sb = const.tile([P, P], f16)
    nc.sync.dma_start(iota_sb[:], io["iota"][:])
    dstv_sb = const.tile([P, NSLOT], f32)
    nc.sync.dma_start(dstv_sb[:], io["dstv"][:])
    w1_sb = const.tile([P, NSLOT], f32)
    nc.sync.dma_start(w1_sb[:], io["w1"][:])
    w2_sb = const.tile([P, NSLOT], f32)
    nc.sync.dma_start(w2_sb[:], io["w2"][:])

    xte_sb = const.tile([P, NSLOT * P], f16)
    for lo in range(0, NSLOT, ECHUNK):
        hi = min(lo + ECHUNK, NSLOT)
        nc.sync.dma_start(xte_sb[:, lo * P:hi * P], io["xte"][:, lo * P:hi * P])

    xt_sb = const.tile([P, NC_NODES], f16)
    for lo in range(0, NC_NODES, XCHUNK):
        hi = min(lo + XCHUNK, NC_NODES)
        nc.sync.dma_start(xt_sb[:, lo:hi], io["xt"][:, lo:hi])

    # phase B: raw messages for every slot (h of both graphs, fp16)
    msg_sb = const.tile([P, NSLOT, 2 * C], f16)
    for s in range(NSLOT):
        pm = psm.tile([P, 2 * C], f32)
        nc.tensor.matmul(
            pm[:], lhsT=xte_sb[:, s * P:(s + 1) * P], rhs=W_sb[:, C:3 * C],
            start=True, stop=True,
        )
        nc.scalar.activation(
            out=msg_sb[:, s, :], in_=pm[:], func=mybir.ActivationFunctionType.Copy,
        )

    # phase C: per out-tile dense + merge accumulation in PSUM
    for t, slots in cfg["tile_slots"]:
        merges = []  # (lhsT, rhs) for this tile's psum group
        for s in slots:
            s1 = spool.tile([P, P], f16, tag="sel")
            nc.vector.tensor_scalar(
                out=s1[:], in0=iota_sb[:], scalar1=dstv_sb[:, s:s + 1],
                scalar2=w1_sb[:, s:s + 1],
                op0=mybir.AluOpType.is_equal, op1=mybir.AluOpType.mult,
            )
            merges.append((s1, msg_sb[:, s, 0:C]))
            s2 = spool.tile([P, P], f16, tag="sel")
            nc.vector.tensor_scalar(
                out=s2[:], in0=iota_sb[:], scalar1=dstv_sb[:, s:s + 1],
                scalar2=w2_sb[:, s:s + 1],
                op0=mybir.AluOpType.is_equal, op1=mybir.AluOpType.mult,
            )
            merges.append((s2, msg_sb[:, s, C:2 * C]))

        pt = ps.tile([P, C], f32)
        nc.tensor.matmul(
            pt[:], lhsT=xt_sb[:, t * P:(t + 1) * P], rhs=W_sb[:, 0:C],
            start=True, stop=False,
        )
        for i, (sel, rhs) in enumerate(merges):
            nc.tensor.matmul(
                pt[:], lhsT=sel[:], rhs=rhs,
                start=False, stop=(i == len(merges) - 1),
            )
        o_sb = osb.tile([P, C], f32)
        nc.vector.tensor_add(o_sb[:], pt[:], shift_sb[:])
        nc.sync.dma_start(out_d[t * P:(t + 1) * P, :], o_sb[:])


def _build(cfg):
    nc = bacc.Bacc("TRN2", target_bir_lowering=False, debug=False)
    NSLOT = cfg["NSLOT"]
    f16 = mybir.dt.float16
    f32 = mybir.dt.float32
    io = {}
    for name, shape, dt in [
        ("xt", [P, NC_NODES], f16),
        ("xte", [P, NSLOT * P], f16),
        ("wcat", [P, 3 * C], f16),
        ("shift", [P, C], f32),
        ("iota", [P, P], f16),
        ("dstv", [P, NSLOT], f32),
        ("w1", [P, NSLOT], f32),
        ("w2", [P, NSLOT], f32),
    ]:
        io[name] = nc.dram_tensor(name, shape, dt, kind="ExternalInput").ap()
    io["out"] = nc.dram_tensor("out", [NC_NODES, C], f32,
                               kind="ExternalOutput").ap()
    with tile.TileContext(nc) as tc:
        _emit(tc, io, cfg)
    nc.compile()
    return nc


def kernel(_trace=False, _sim_core=None, **inputs) -> np.ndarray:
    in_maps, cfg = _prep(**inputs)
    nc = _build(cfg)

    if _sim_core is not None:
        from concourse.bass_interp import CoreSim
        sim = CoreSim(nc, trace=False)
        for k, v in in_maps[_sim_core].items():
            sim.tensor(k)[:] = v
        sim.tensor("out")[:] = 0.0
        sim.simulate(check_with_hw=False)
        return np.array(sim.tensor("out"))

    res = run_bass_kernel_spmd(
        nc, in_maps, core_ids=list(range(NCORES)),
        trace=_trace, trace_cores=[0] if _trace else None,
    )
    out = np.empty((NPAD, C), np.float32)
    for c in range(NCORES):
        out[c * NC_NODES:(c + 1) * NC_NODES] = res.results[c]["out"][:NC_NODES]
    if _trace:
        kernel.last_exec_time_ns = res.exec_time_ns
        kernel.last_results = res
    return out[:N]


# revision 4
# speedup vs baseline: 3.9756x; 1.8171x over previous
"""DiGCN_IB_1BN kernel for Trainium2 (8 NeuronCores, SPMD data-parallel).

Math (see reference):
  out = BN(x @ Wl + bl + conv1 + conv2)
  conv_g = segment_sum((x @ Wg)[src] * w, dst) + bg, edges masked to
  same-1024-block pairs only.

Strategy (v3):
  - BN + biases folded on host into per-channel scale (inside the f16 W mats)
    and one additive f32 shift; edge weights folded into the token features
    (xe_g column j = w_j * x[src_j], zero if token j isn't graph g).
  - Nodes sharded across 8 cores by contiguous 13-block groups (13312
    nodes/core), zero cross-core communication. All matmul inputs fp16, PSUM
    accumulates fp32.
  - Node interleave permutation: within each 1024-node group, MM-tile s
    (0..7) owns nodes {base + p*8 + s}. Out-tiles then store as one
    [128, 8, 64] DMA per group with 2KB contiguous DRAM runs (13 stores
    instead of 104).
  - Tokens (surviving edges) grouped by destination tile; one 128-token slot
    per tile (2 on rare overflow). Fully on-chip per slot:
      msg:  psum_m[:, i, :] = xe1_slot.T @ W1' + xe2_slot.T @ W2'
            (token-major [128,64], both graphs via disjoint-zero xe halves)
      S:    S[k, m] = (dstv[k] == m), one banded tensor_tensor(is_equal)
            over 4 slots at a time (dstv broadcast vs constant iota).
      out:  psum_t = xt_tile.T @ Wl' + S_slot.T @ msg_slot  (PSUM accum)
      store: og[:, s, :] = psum_t + shift (DVE tensor_tensor) -> group DMA.
  No indirect/scatter DMA anywhere (v1's dma_scatter_add measured ~7ns/token
  of serialized Q7 descriptor-gen; v2's per-slot tensor_scalar S-builds and
  104 single-tile stores dominated DVE/sync engines).
"""

import sys

sys.path.insert(0, "/opt/trn_rl_repo")

from contextlib import ExitStack

import numpy as np

import concourse.bass as bass
import concourse.tile as tile
from concourse import bacc, mybir
from concourse._compat import with_exitstack
from concourse.bass_utils import run_bass_kernel_spmd

# problem constants (hardcoded per harness contract)
N = 100000
F = 128
C = 64
BS = 1024
EPS = 1e-5
NCORES = 8
BPC = 13  # 1024-node groups per core
NC_NODES = BPC * BS  # 13312
NPAD = NCORES * NC_NODES  # 106496
P = 128
NTILES = NC_NODES // P  # 104
BAND = 4  # slots per S-build / msg-copy band


def _prep(x, edge_index, edge_weight, edge_index2, edge_weight2,
          Wl, bl, W1, b1, W2, b2, gamma, beta, run_mean, run_var):
    """Host-side sharding + layout. Returns (in_maps, cfg)."""
    inv = (gamma / np.sqrt(run_var + EPS)).astype(np.float32)
    Wcat = np.concatenate(
        [Wl * inv[None, :], W1 * inv[None, :], W2 * inv[None, :]], axis=1
    ).astype(np.float16)  # [128, 192]
    shift = ((bl + b1 + b2 - run_mean) * inv + beta).astype(np.float32)
    shift_rep = np.ascontiguousarray(np.tile(shift[None, :], (P, 1)))
    iota_rep = np.ascontiguousarray(
        np.tile(np.arange(P, dtype=np.float32)[None, :], (P, BAND))
    )  # [128, BAND*128]

    xpad = np.zeros((NPAD, F), np.float32)
    xpad[:N] = x

    # node interleave permutation: column q = t*128 + p of xt holds node
    # (t//8)*1024 + p*8 + (t%8) (core-local)
    q = np.arange(NC_NODES)
    tq, pq = q // P, q % P
    node_of_q = (tq // 8) * 1024 + pq * 8 + (tq % 8)

    # per-core, per-graph surviving edges -> (tile, p, src, w)
    per_core = [[None, None] for _ in range(NCORES)]
    for g, (ei, ew) in enumerate([(edge_index, edge_weight),
                                  (edge_index2, edge_weight2)]):
        src = np.asarray(ei[0], dtype=np.int64)
        dst = np.asarray(ei[1], dtype=np.int64)
        keep = (src // BS) == (dst // BS)
        src = src[keep]
        dst = dst[keep]
        w = np.asarray(ew, dtype=np.float32)[keep]
        core = dst // NC_NODES
        for c in range(NCORES):
            m = core == c
            dl = dst[m] - c * NC_NODES
            r = dl % BS
            tile_id = (dl // BS) * 8 + (r % 8)
            per_core[c][g] = (src[m], tile_id, r // 8, w[m])

    # slot sizing: counts per (core, tile) over both graphs
    counts = np.zeros((NCORES, NTILES), np.int64)
    for c in range(NCORES):
        for g in range(2):
            np.add.at(counts[c], per_core[c][g][1], 1)
    slots_per_tile = np.maximum(1, -(-counts.max(axis=0) // P))
    slot0 = np.concatenate([[0], np.cumsum(slots_per_tile)])
    NSLOT = int(slot0[-1])

    in_maps = []
    for c in range(NCORES):
        src_all = np.concatenate([per_core[c][0][0], per_core[c][1][0]])
        tile_all = np.concatenate([per_core[c][0][1], per_core[c][1][1]])
        p_all = np.concatenate([per_core[c][0][2], per_core[c][1][2]])
        w_all = np.concatenate([per_core[c][0][3], per_core[c][1][3]])
        gr_all = np.concatenate([
            np.zeros(len(per_core[c][0][0]), np.int64),
            np.ones(len(per_core[c][1][0]), np.int64),
        ])
        order = np.argsort(tile_all, kind="stable")
        st = tile_all[order]
        # rank within tile
        starts = np.searchsorted(st, np.arange(NTILES), side="left")
        rank = np.arange(len(st)) - starts[st]
        j = slot0[st] * P + rank  # token position
        assert (rank < slots_per_tile[st] * P).all()

        ntok = NSLOT * P
        src_tok = np.zeros(ntok, np.int64)
        w1t = np.zeros(ntok, np.float32)
        w2t = np.zeros(ntok, np.float32)
        dstv = np.zeros(ntok, np.float32)
        src_tok[j] = src_all[order]
        dstv[j] = p_all[order].astype(np.float32)
        g_ord = gr_all[order]
        w1t[j[g_ord == 0]] = w_all[order][g_ord == 0]
        w2t[j[g_ord == 1]] = w_all[order][g_ord == 1]

        xsrc = xpad[src_tok]  # [ntok, 128] f32 (global src index)
        xe1 = np.ascontiguousarray((xsrc * w1t[:, None]).astype(np.float16).T)
        xe2 = np.ascontiguousarray((xsrc * w2t[:, None]).astype(np.float16).T)
        dstv_c = np.ascontiguousarray(dstv.reshape(NSLOT, P).T)  # [128, NSLOT]
        xt = np.ascontiguousarray(
            xpad[c * NC_NODES + node_of_q].astype(np.float16).T)

        in_maps.append({
            "xt": xt,            # [128, 13312] f16 (interleave-permuted)
            "xe1": xe1,          # [128, NSLOT*128] f16
            "xe2": xe2,          # [128, NSLOT*128] f16
            "wcat": Wcat,        # [128, 192] f16
            "shift": shift_rep,  # [128, 64] f32
            "iota": iota_rep,    # [128, BAND*128] f32
            "dstv": dstv_c,      # [128, NSLOT] f32
        })

    cfg = {"NSLOT": NSLOT,
           "tile_slots": [(t, list(range(int(slot0[t]), int(slot0[t + 1]))))
                          for t in range(NTILES)]}
    return in_maps, cfg


@with_exitstack
def _emit(ctx: ExitStack, tc: tile.TileContext, io, cfg):
    nc = tc.nc
    out_d = io["out"]
    NSLOT = cfg["NSLOT"]
    f16 = mybir.dt.float16
    f32 = mybir.dt.float32
    XCHUNK = 26 * P  # 3328 cols (~0.85MB f16) per xt/xe load chunk

    const = ctx.enter_context(tc.tile_pool(name="const", bufs=1))
    ogp = ctx.enter_context(tc.tile_pool(name="ogp", bufs=3))
    ps = ctx.enter_context(tc.tile_pool(name="ps", bufs=4, space="PSUM"))
    psm = ctx.enter_context(tc.tile_pool(name="psm", bufs=3, space="PSUM"))

    W_sb = const.tile([P, 3 * C], f16)
    nc.sync.dma_start(W_sb[:], io["wcat"][:])
    shift_sb = const.tile([P, C], f32)
    nc.sync.dma_start(shift_sb[:], io["shift"][:])
    iota_sb = const.tile([P, BAND, P], f32)
    nc.sync.dma_start(iota_sb[:, :, :], io["iota"][:, :])
    dstv_sb = const.tile([P, NSLOT], f32)
    nc.sync.dma_start(dstv_sb[:], io["dstv"][:])

    xe1_sb = const.tile([P, NSLOT * P], f16)
    xe2_sb = const.tile([P, NSLOT * P], f16)
    for lo in range(0, NSLOT * P, XCHUNK):
        hi = min(lo + XCHUNK, NSLOT * P)
        nc.sync.dma_start(xe1_sb[:, lo:hi], io["xe1"][:, lo:hi])
        nc.sync.dma_start(xe2_sb[:, lo:hi], io["xe2"][:, lo:hi])
    xt_sb = const.tile([P, NC_NODES], f16)
    for lo in range(0, NC_NODES, XCHUNK):
        hi = min(lo + XCHUNK, NC_NODES)
        nc.sync.dma_start(xt_sb[:, lo:hi], io["xt"][:, lo:hi])

    msg_all = const.tile([P, NSLOT, C], f16)
    S_all = const.tile([P, NSLOT, P], f16)

    # phase B: messages (PE) + S matrices (DVE), banded
    for b0 in range(0, NSLOT, BAND):
        k = min(BAND, NSLOT - b0)
        pm = psm.tile([P, BAND, C], f32)
        for i in range(k):
            s = b0 + i
            nc.tensor.matmul(
                pm[:, i, :], lhsT=xe1_sb[:, s * P:(s + 1) * P],
                rhs=W_sb[:, C:2 * C], start=True, stop=False,
                skip_group_check=True,
            )
            nc.tensor.matmul(
                pm[:, i, :], lhsT=xe2_sb[:, s * P:(s + 1) * P],
                rhs=W_sb[:, 2 * C:3 * C], start=False, stop=True,
                skip_group_check=True,
            )
        nc.scalar.activation(
            out=msg_all[:, b0:b0 + k, :], in_=pm[:, 0:k, :],
            func=mybir.ActivationFunctionType.Copy,
        )
        nc.vector.tensor_tensor(
            out=S_all[:, b0:b0 + k, :],
            in0=dstv_sb[:, b0:b0 + k].to_broadcast([P, k, P]),
            in1=iota_sb[:, 0:k, :],
            op=mybir.AluOpType.is_equal,
        )

    # phase C: per-tile dense + merge in PSUM; grouped stores
    og = None
    for t, slots in cfg["tile_slots"]:
        G, s_sub = t // 8, t % 8
        if s_sub == 0:
            og = ogp.tile([P, 8, C], f32)
        pt = ps.tile([P, C], f32)
        nc.tensor.matmul(
            pt[:], lhsT=xt_sb[:, t * P:(t + 1) * P], rhs=W_sb[:, 0:C],
            start=True, stop=False, skip_group_check=True,
        )
        for i, s in enumerate(slots):
            nc.tensor.matmul(
                pt[:], lhsT=S_all[:, s, :], rhs=msg_all[:, s, :],
                start=False, stop=(i == len(slots) - 1),
                skip_group_check=True,
            )
        nc.vector.tensor_add(og[:, s_sub, :], pt[:], shift_sb[:])
        if s_sub == 7:
            nc.sync.dma_start(
                out_d[G * BS:(G + 1) * BS, :].rearrange(
                    "(p s) c -> p s c", s=8),
                og[:, :, :],
            )


def _build(cfg):
    nc = bacc.Bacc("TRN2", target_bir_lowering=False, debug=False)
    NSLOT = cfg["NSLOT"]
    f16 = mybir.dt.float16
    f32 = mybir.dt.float32
    io = {}
    for name, shape, dt in [
        ("xt", [P, NC_NODES], f16),
        ("xe1", [P, NSLOT * P], f16),
        ("xe2", [P, NSLOT * P], f16),
        ("wcat", [P, 3 * C], f16),
        ("shift", [P, C], f32),
        ("iota", [P, BAND * P], f32),
        ("dstv", [P, NSLOT], f32),
    ]:
        io[name] = nc.dram_tensor(name, shape, dt, kind="ExternalInput").ap()
    io["out"] = nc.dram_tensor("out", [NC_NODES, C], f32,
                               kind="ExternalOutput").ap()
    with tile.TileContext(nc) as tc:
        _emit(tc, io, cfg)
    nc.compile()
    return nc


def kernel(_trace=False, _sim_core=None, **inputs) -> np.ndarray:
    in_maps, cfg = _prep(**inputs)
    nc = _build(cfg)

    if _sim_core is not None:
        from concourse.bass_interp import CoreSim
        sim = CoreSim(nc, trace=False)
        for k, v in in_maps[_sim_core].items():
            sim.tensor(k)[:] = v
        sim.tensor("out")[:] = 0.0
        sim.simulate(check_with_hw=False)
        return np.array(sim.tensor("out"))

    res = run_bass_kernel_spmd(
        nc, in_maps, core_ids=list(range(NCORES)),
        trace=_trace, trace_cores=[0] if _trace else None,
    )
    out = np.empty((NPAD, C), np.float32)
    for c in range(NCORES):
        out[c * NC_NODES:(c + 1) * NC_NODES] = res.results[c]["out"][:NC_NODES]
    if _trace:
        kernel.last_exec_time_ns = res.exec_time_ns
        kernel.last_results = res
    return out[:N]


# revision 5
# speedup vs baseline: 3.9759x; 1.0001x over previous
"""DiGCN_IB_1BN kernel for Trainium2 (8 NeuronCores, SPMD data-parallel).

Math (see reference):
  out = BN(x @ Wl + bl + conv1 + conv2)
  conv_g = segment_sum((x @ Wg)[src] * w, dst) + bg, edges masked to
  same-1024-block pairs only.

Strategy (v3):
  - BN + biases folded on host into per-channel scale (inside the f16 W mats)
    and one additive f32 shift; edge weights folded into the token features
    (xe_g column j = w_j * x[src_j], zero if token j isn't graph g).
  - Nodes sharded across 8 cores by contiguous 13-block groups (13312
    nodes/core), zero cross-core communication. All matmul inputs fp16, PSUM
    accumulates fp32.
  - Node interleave permutation: within each 1024-node group, MM-tile s
    (0..7) owns nodes {base + p*8 + s}. Out-tiles then store as one
    [128, 8, 64] DMA per group with 2KB contiguous DRAM runs (13 stores
    instead of 104).
  - Tokens (surviving edges) grouped by destination tile; one 128-token slot
    per tile (2 on rare overflow). Fully on-chip per slot:
      msg:  psum_m[:, i, :] = xe1_slot.T @ W1' + xe2_slot.T @ W2'
            (token-major [128,64], both graphs via disjoint-zero xe halves)
      S:    S[k, m] = (dstv[k] == m), one banded tensor_tensor(is_equal)
            over 4 slots at a time (dstv broadcast vs constant iota).
      out:  psum_t = xt_tile.T @ Wl' + S_slot.T @ msg_slot  (PSUM accum)
      store: og[:, s, :] = psum_t + shift (DVE tensor_tensor) -> group DMA.
  No indirect/scatter DMA anywhere (v1's dma_scatter_add measured ~7ns/token
  of serialized Q7 descriptor-gen; v2's per-slot tensor_scalar S-builds and
  104 single-tile stores dominated DVE/sync engines).
"""

import sys

sys.path.insert(0, "/opt/trn_rl_repo")

from contextlib import ExitStack

import numpy as np

import concourse.bass as bass
import concourse.tile as tile
from concourse import bacc, mybir
from concourse._compat import with_exitstack
from concourse.bass_utils import run_bass_kernel_spmd

# problem constants (hardcoded per harness contract)
N = 100000
F = 128
C = 64
BS = 1024
EPS = 1e-5
NCORES = 8
BPC = 13  # 1024-node groups per core
NC_NODES = BPC * BS  # 13312
NPAD = NCORES * NC_NODES  # 106496
P = 128
NTILES = NC_NODES // P  # 104
BAND = 4  # slots per S-build / msg-copy band


def _prep(x, edge_index, edge_weight, edge_index2, edge_weight2,
          Wl, bl, W1, b1, W2, b2, gamma, beta, run_mean, run_var):
    """Host-side sharding + layout. Returns (in_maps, cfg)."""
    inv = (gamma / np.sqrt(run_var + EPS)).astype(np.float32)
    Wcat = np.concatenate(
        [Wl * inv[None, :], W1 * inv[None, :], W2 * inv[None, :]], axis=1
    ).astype(np.float16)  # [128, 192]
    shift = ((bl + b1 + b2 - run_mean) * inv + beta).astype(np.float32)
    shift_rep = np.ascontiguousarray(np.tile(shift[None, :], (P, 2)))
    iota_rep = np.ascontiguousarray(
        np.tile(np.arange(P, dtype=np.float16)[None, :], (P, BAND))
    )  # [128, BAND*128]

    xpad = np.zeros((NPAD, F), np.float32)
    xpad[:N] = x

    # node interleave permutation: column q = t*128 + p of xt holds node
    # (t//8)*1024 + p*8 + (t%8) (core-local)
    q = np.arange(NC_NODES)
    tq, pq = q // P, q % P
    node_of_q = (tq // 8) * 1024 + pq * 8 + (tq % 8)

    # per-core, per-graph surviving edges -> (tile, p, src, w)
    per_core = [[None, None] for _ in range(NCORES)]
    for g, (ei, ew) in enumerate([(edge_index, edge_weight),
                                  (edge_index2, edge_weight2)]):
        src = np.asarray(ei[0], dtype=np.int64)
        dst = np.asarray(ei[1], dtype=np.int64)
        keep = (src // BS) == (dst // BS)
        src = src[keep]
        dst = dst[keep]
        w = np.asarray(ew, dtype=np.float32)[keep]
        core = dst // NC_NODES
        for c in range(NCORES):
            m = core == c
            dl = dst[m] - c * NC_NODES
            r = dl % BS
            tile_id = (dl // BS) * 8 + (r % 8)
            per_core[c][g] = (src[m], tile_id, r // 8, w[m])

    # slot sizing: counts per (core, tile) over both graphs
    counts = np.zeros((NCORES, NTILES), np.int64)
    for c in range(NCORES):
        for g in range(2):
            np.add.at(counts[c], per_core[c][g][1], 1)
    slots_per_tile = np.maximum(1, -(-counts.max(axis=0) // P))
    slot0 = np.concatenate([[0], np.cumsum(slots_per_tile)])
    NSLOT = int(slot0[-1])

    in_maps = []
    for c in range(NCORES):
        src_all = np.concatenate([per_core[c][0][0], per_core[c][1][0]])
        tile_all = np.concatenate([per_core[c][0][1], per_core[c][1][1]])
        p_all = np.concatenate([per_core[c][0][2], per_core[c][1][2]])
        w_all = np.concatenate([per_core[c][0][3], per_core[c][1][3]])
        gr_all = np.concatenate([
            np.zeros(len(per_core[c][0][0]), np.int64),
            np.ones(len(per_core[c][1][0]), np.int64),
        ])
        order = np.argsort(tile_all, kind="stable")
        st = tile_all[order]
        # rank within tile
        starts = np.searchsorted(st, np.arange(NTILES), side="left")
        rank = np.arange(len(st)) - starts[st]
        j = slot0[st] * P + rank  # token position
        assert (rank < slots_per_tile[st] * P).all()

        ntok = NSLOT * P
        src_tok = np.zeros(ntok, np.int64)
        w1t = np.zeros(ntok, np.float32)
        w2t = np.zeros(ntok, np.float32)
        dstv = np.zeros(ntok, np.float16)
        src_tok[j] = src_all[order]
        dstv[j] = p_all[order].astype(np.float16)
        g_ord = gr_all[order]
        w1t[j[g_ord == 0]] = w_all[order][g_ord == 0]
        w2t[j[g_ord == 1]] = w_all[order][g_ord == 1]

        xsrc = xpad[src_tok]  # [ntok, 128] f32 (global src index)
        xe1 = np.ascontiguousarray((xsrc * w1t[:, None]).astype(np.float16).T)
        xe2 = np.ascontiguousarray((xsrc * w2t[:, None]).astype(np.float16).T)
        dstv_c = np.ascontiguousarray(dstv.reshape(NSLOT, P).T)  # [128, NSLOT]
        xt = np.ascontiguousarray(
            xpad[c * NC_NODES + node_of_q].astype(np.float16).T)

        in_maps.append({
            "xt": xt,            # [128, 13312] f16 (interleave-permuted)
            "xe1": xe1,          # [128, NSLOT*128] f16
            "xe2": xe2,          # [128, NSLOT*128] f16
            "wcat": Wcat,        # [128, 192] f16
            "shift": shift_rep,  # [128, 128] f32
            "iota": iota_rep,    # [128, BAND*128] f16
            "dstv": dstv_c,      # [128, NSLOT] f16
        })

    cfg = {"NSLOT": NSLOT,
           "tile_slots": [(t, list(range(int(slot0[t]), int(slot0[t + 1]))))
                          for t in range(NTILES)]}
    return in_maps, cfg


@with_exitstack
def _emit(ctx: ExitStack, tc: tile.TileContext, io, cfg):
    nc = tc.nc
    out_d = io["out"]
    NSLOT = cfg["NSLOT"]
    f16 = mybir.dt.float16
    f32 = mybir.dt.float32
    XCHUNK = 26 * P  # 3328 cols (~0.85MB f16) per xt/xe load chunk

    const = ctx.enter_context(tc.tile_pool(name="const", bufs=1))
    ogp = ctx.enter_context(tc.tile_pool(name="ogp", bufs=3))
    ps = ctx.enter_context(tc.tile_pool(name="ps", bufs=4, space="PSUM"))
    psm = ctx.enter_context(tc.tile_pool(name="psm", bufs=3, space="PSUM"))

    W_sb = const.tile([P, 3 * C], f16)
    nc.sync.dma_start(W_sb[:], io["wcat"][:])
    shift_sb = const.tile([P, 2, C], f32)
    nc.sync.dma_start(shift_sb[:, :, :], io["shift"][:])
    iota_sb = const.tile([P, BAND, P], f16)
    nc.sync.dma_start(iota_sb[:, :, :], io["iota"][:, :])
    dstv_sb = const.tile([P, NSLOT], f16)
    nc.sync.dma_start(dstv_sb[:], io["dstv"][:])

    xe1_sb = const.tile([P, NSLOT * P], f16)
    xe2_sb = const.tile([P, NSLOT * P], f16)
    xt_sb = const.tile([P, NC_NODES], f16)
    for lo in range(0, NSLOT * P, XCHUNK):
        hi = min(lo + XCHUNK, NSLOT * P)
        nc.sync.dma_start(xe1_sb[:, lo:hi], io["xe1"][:, lo:hi])
        nc.scalar.dma_start(xe2_sb[:, lo:hi], io["xe2"][:, lo:hi])
        xlo = lo * NC_NODES // (NSLOT * P)
        xhi = min(NC_NODES, hi * NC_NODES // (NSLOT * P))
        xlo, xhi = (xlo // P) * P, (xhi // P) * P
        if xhi > xlo:
            nc.sync.dma_start(xt_sb[:, xlo:xhi], io["xt"][:, xlo:xhi])
    if (NC_NODES // P) * P < NC_NODES:
        pass

    msg_all = const.tile([P, NSLOT, C], f16)
    S_all = const.tile([P, NSLOT, P], f16)

    # phase B: messages (PE) + S matrices (DVE), banded
    for b0 in range(0, NSLOT, BAND):
        k = min(BAND, NSLOT - b0)
        pm = psm.tile([P, BAND, C], f32)
        for i in range(k):
            s = b0 + i
            nc.tensor.matmul(
                pm[:, i, :], lhsT=xe1_sb[:, s * P:(s + 1) * P],
                rhs=W_sb[:, C:2 * C], start=True, stop=False,
                skip_group_check=True,
            )
            nc.tensor.matmul(
                pm[:, i, :], lhsT=xe2_sb[:, s * P:(s + 1) * P],
                rhs=W_sb[:, 2 * C:3 * C], start=False, stop=True,
                skip_group_check=True,
            )
        nc.scalar.activation(
            out=msg_all[:, b0:b0 + k, :], in_=pm[:, 0:k, :],
            func=mybir.ActivationFunctionType.Copy,
        )
        nc.vector.tensor_tensor(
            out=S_all[:, b0:b0 + k, :],
            in0=dstv_sb[:, b0:b0 + k].to_broadcast([P, k, P]),
            in1=iota_sb[:, 0:k, :],
            op=mybir.AluOpType.is_equal,
        )

    # phase C: per-tile dense + merge in PSUM; paired TT, grouped stores
    og = None
    ts_list = cfg["tile_slots"]
    for ti in range(0, NTILES, 2):
        pt = ps.tile([P, 2, C], f32)
        for half in range(2):
            t, slots = ts_list[ti + half]
            G, s_sub = t // 8, t % 8
            if s_sub == 0:
                og = ogp.tile([P, 8, C], f32)
            nc.tensor.matmul(
                pt[:, half, :], lhsT=xt_sb[:, t * P:(t + 1) * P],
                rhs=W_sb[:, 0:C],
                start=True, stop=False, skip_group_check=True,
            )
            for i, s in enumerate(slots):
                nc.tensor.matmul(
                    pt[:, half, :], lhsT=S_all[:, s, :], rhs=msg_all[:, s, :],
                    start=False, stop=(i == len(slots) - 1),
                    skip_group_check=True,
                )
        nc.vector.tensor_add(og[:, s_sub - 1:s_sub + 1, :], pt[:, :, :],
                             shift_sb[:, :, :])
        if s_sub == 7:
            nc.sync.dma_start(
                out_d[G * BS:(G + 1) * BS, :].rearrange(
                    "(p s) c -> p s c", s=8),
                og[:, :, :],
            )


def _build(cfg):
    nc = bacc.Bacc("TRN2", target_bir_lowering=False, debug=False)
    NSLOT = cfg["NSLOT"]
    f16 = mybir.dt.float16
    f32 = mybir.dt.float32
    io = {}
    for name, shape, dt in [
        ("xt", [P, NC_NODES], f16),
        ("xe1", [P, NSLOT * P], f16),
        ("xe2", [P, NSLOT * P], f16),
        ("wcat", [P, 3 * C], f16),
        ("shift", [P, 2 * C], f32),
        ("iota", [P, BAND * P], f16),
        ("dstv", [P, NSLOT], f16),
    ]:
        io[name] = nc.dram_tensor(name, shape, dt, kind="ExternalInput").ap()
    io["out"] = nc.dram_tensor("out", [NC_NODES, C], f32,
                               kind="ExternalOutput").ap()
    with tile.TileContext(nc) as tc:
        _emit(tc, io, cfg)
    nc.compile()
    return nc


def kernel(_trace=False, _sim_core=None, **inputs) -> np.ndarray:
    in_maps, cfg = _prep(**inputs)
    nc = _build(cfg)

    if _sim_core is not None:
        from concourse.bass_interp import CoreSim
        sim = CoreSim(nc, trace=False)
        for k, v in in_maps[_sim_core].items():
            sim.tensor(k)[:] = v
        sim.tensor("out")[:] = 0.0
        sim.simulate(check_with_hw=False)
        return np.array(sim.tensor("out"))

    res = run_bass_kernel_spmd(
        nc, in_maps, core_ids=list(range(NCORES)),
        trace=_trace, trace_cores=[0] if _trace else None,
    )
    out = np.empty((NPAD, C), np.float32)
    for c in range(NCORES):
        out[c * NC_NODES:(c + 1) * NC_NODES] = res.results[c]["out"][:NC_NODES]
    if _trace:
        kernel.last_exec_time_ns = res.exec_time_ns
        kernel.last_results = res
    return out[:N]
